# revision 9
# baseline (speedup 1.0000x reference)
"""Trainium2 Bass kernel for nn_AttentionBlock (GroupNorm32 + 4-head self
attention over 64x64 spatial + output projection + residual).

Sharding over 8 NeuronCores: core = (sample s, head-group hg) with
s = core // 2 in [0,4), hg = core % 2 selecting global heads {2*hg, 2*hg+1}.
Each core: groupnorm(sample) -> QKV for its two heads -> attention computed
entirely in a transposed layout (scores S^T[j,i] so softmax reductions ride
on the TensorEngine) -> partial projection over its 128 output channels'
contraction slice. Host sums the two partials per sample and adds the
residual, proj bias, and the constant v-bias correction proj_w[:,shard] @ bv.

dtype plan: the large matmuls (QKV, scores, projection) run in float32r
(PE full speed at N>=512; producers write f32r directly so the operands are
hardware-rounded). The post-softmax AV matmul runs in bf16 (exp emits bf16;
errors there don't get amplified by the softmax). Tiny stats/broadcast
matmuls run in plain fp32. Softmax needs no max-subtraction: scores/8 are
~N(0,1) here; denominators come from a ones-augmented AV matmul column.
"""
import numpy as np

NUM_GROUPS = 32
EPS = 1e-5
B, C, Hs, Ws = 4, 256, 64, 64
NHEADS = 4
D = C // NHEADS          # 64
HW = Hs * Ws             # 4096
N_CORES = 8
NI = 512                 # i-chunk (query positions per chunk)
NIC = HW // NI           # 8 i-chunks
NJ = HW // 128           # 32 j-tiles (key positions)

_cache = {}


def _build_module():
    from contextlib import ExitStack
    import concourse.bass as bass
    import concourse.tile as tile
    from concourse import bacc, mybir

    f32 = mybir.dt.float32
    f32r = mybir.dt.float32r
    bf16 = mybir.dt.bfloat16
    ALU = mybir.AluOpType
    ACTF = mybir.ActivationFunctionType
    ts = bass.ts

    nc = bacc.Bacc("TRN2", target_bir_lowering=False, debug=False,
                   num_devices=N_CORES)

    def din(name, shape):
        return nc.dram_tensor(name, shape, f32, kind="ExternalInput").ap()

    x_d = din("x_s", [C, HW])
    wq_d = din("wq", [128, 256])
    wk_d = din("wk", [128, 256])
    wv_d = din("wv", [128, 256])
    wp_d = din("wp", [128, 256])
    bq_d = din("bq", [128, 1])
    bk_d = din("bk", [128, 1])
    gnsc_d = din("gnsc", [128, 2])
    gnbi_d = din("gnbi", [128, 2])
    sel_d = din("sel", [128, 64])
    rep_d = din("rep", [32, 256])
    e16_d = din("e16", [16, 2048])
    out_d = nc.dram_tensor("outp", [C, HW], f32, kind="ExternalOutput").ap()

    with tile.TileContext(nc) as tc, ExitStack() as ctx:
        persist = ctx.enter_context(tc.tile_pool(name="persist", bufs=1))

        # ---- long-lived tiles ----
        qTr = persist.tile([128, HW], f32r, tag="qTr")
        kTr = persist.tile([128, HW], f32r, tag="kTr")
        # v_aug layout per (j,h): col 0 = ones (softmax denominator row),
        # cols 1-63 = zeros, cols 64-127 = v. The AV matmul then emits sums
        # at PSUM row 0 (partition-aligned copy out) and out values at rows
        # 64-127 (64-row quadrant-pair move, HW-verified).
        v_aug = persist.tile([128, NJ, 2, 128], bf16, tag="vaug")
        outT = persist.tile([128, HW], f32, tag="outT")
        out_norm = persist.tile([128, HW], f32r, tag="out_norm")
        sums16 = persist.tile([16, NI], f32, tag="sums16")
        recip16 = persist.tile([16, NI], f32, tag="recip16")
        e16 = persist.tile([16, 2048], f32, tag="e16")
        nc.sync.dma_start(e16[:], e16_d)
        wpr = persist.tile([128, 256], f32r, tag="wpr")
        bq = persist.tile([128, 1], f32, tag="bq")
        nc.sync.dma_start(bq[:], bq_d)
        bk = persist.tile([128, 1], f32, tag="bk")
        nc.sync.dma_start(bk[:], bk_d)

        # ---- early phase: loads, groupnorm, QKV ----
        with tc.tile_pool(name="early", bufs=1) as early, \
             tc.tile_pool(name="gnps", bufs=1, space="PSUM") as gnps:
            x0 = early.tile([128, HW], f32, tag="x0")
            x1 = early.tile([128, HW], f32, tag="x1")
            nc.sync.dma_start(x0[:], x_d[0:128, :])
            nc.sync.dma_start(x1[:], x_d[128:256, :])
            xt = [x0, x1]
            xnr0 = early.tile([128, HW], f32r, tag="xnr0")
            xnr1 = early.tile([128, HW], f32r, tag="xnr1")
            xnr = [xnr0, xnr1]
            wstage = early.tile([128, 4, 256], f32, tag="wstage")
            for i, wd in enumerate((wq_d, wk_d, wv_d, wp_d)):
                nc.sync.dma_start(wstage[:, i, :], wd)
            wqr = early.tile([128, 256], f32r, tag="wqr")
            wkr = early.tile([128, 256], f32r, tag="wkr")
            wvr = early.tile([128, 256], f32r, tag="wvr")
            for i, wr_t in enumerate((wqr, wkr, wvr, wpr)):
                nc.vector.tensor_copy(wr_t[:], wstage[:, i, :])
            sel = early.tile([128, 64], f32, tag="sel")
            nc.sync.dma_start(sel[:], sel_d)
            rep = early.tile([32, 256], f32, tag="rep")
            nc.sync.dma_start(rep[:], rep_d)
            gnsc = early.tile([128, 2], f32, tag="gnsc")
            gnbi = early.tile([128, 2], f32, tag="gnbi")
            nc.sync.dma_start(gnsc[:], gnsc_d)
            nc.sync.dma_start(gnbi[:], gnbi_d)

            # GroupNorm stats via bn_stats/bn_aggr: per-channel (mean, E[x^2])
            stats = [early.tile([128, 2], f32, tag=f"st{c}", name=f"st{c}")
                     for c in (0, 1)]
            for c in (0, 1):
                bnout = early.tile([128, 8, 6], f32, tag="bnout", name="bnout")
                for n in range(8):
                    nc.vector.bn_stats(bnout[:, n, :], xt[c][:, ts(n, 512)])
                nc.vector.bn_aggr(stats[c][:], bnout[:])  # -> (mean, var)
                mt = early.tile([128, 1], f32, tag="mt", name="mt")
                nc.vector.tensor_tensor(out=mt[:], in0=stats[c][:, 0:1],
                                        in1=stats[c][:, 0:1], op=ALU.mult)
                nc.vector.tensor_tensor(out=stats[c][:, 1:2],
                                        in0=stats[c][:, 1:2], in1=mt[:],
                                        op=ALU.add)
            gs_ps = gnps.tile([32, 2], f32, tag="gs")
            nc.tensor.matmul(gs_ps[:], lhsT=sel[:, 0:32], rhs=stats[0][:],
                             start=True, stop=False)
            nc.tensor.matmul(gs_ps[:], lhsT=sel[:, 32:64], rhs=stats[1][:],
                             start=False, stop=True)
            gs = early.tile([32, 2], f32, tag="gs_sb")
            nc.vector.tensor_copy(gs[:], gs_ps[:])
            # gs: col0 = mean_g, col1 = E[x^2]_g   (sel prescaled 1/8)
            rg = early.tile([32, 2], f32, tag="rg")  # col0 rstd, col1 mean
            msq = early.tile([32, 2], f32, tag="msq")
            nc.vector.tensor_copy(rg[:, 1:2], gs[:, 0:1])
            nc.vector.tensor_tensor(out=msq[:, 0:1], in0=gs[:, 0:1],
                                    in1=gs[:, 0:1], op=ALU.mult)
            nc.vector.tensor_tensor(out=msq[:, 1:2], in0=gs[:, 1:2],
                                    in1=msq[:, 0:1], op=ALU.subtract)
            eps_t = early.tile([32, 1], f32, tag="eps")
            nc.vector.memset(eps_t[:], EPS)
            sd = early.tile([32, 1], f32, tag="sd")
            nc.scalar.activation(sd[:], msq[:, 1:2], ACTF.Sqrt, bias=eps_t[:])
            nc.vector.reciprocal(rg[:, 0:1], sd[:])
            for c in (0, 1):
                ab_ps = gnps.tile([128, 2], f32, tag="ab", name="ab")
                nc.tensor.matmul(ab_ps[:], lhsT=rep[:, ts(c, 128)], rhs=rg[:],
                                 start=True, stop=True)
                # A = rstd_ch * gn_scale ; B = gn_bias - mean_ch * A
                AB = early.tile([128, 2], f32, tag=f"ab{c}", name=f"ab{c}")
                nc.vector.tensor_tensor(out=AB[:, 0:1], in0=ab_ps[:, 0:1],
                                        in1=gnsc[:, c:c + 1], op=ALU.mult)
                tmp = early.tile([128, 1], f32, tag=f"tmp{c}", name=f"tmp{c}")
                nc.vector.tensor_tensor(out=tmp[:], in0=ab_ps[:, 1:2],
                                        in1=AB[:, 0:1], op=ALU.mult)
                nc.vector.tensor_tensor(out=AB[:, 1:2], in0=gnbi[:, c:c + 1],
                                        in1=tmp[:], op=ALU.subtract)
                # xn = A*x + B  (written rounded to f32r)
                nc.vector.tensor_scalar(out=xnr[c][:], in0=xt[c][:],
                                        scalar1=AB[:, 0:1], scalar2=AB[:, 1:2],
                                        op0=ALU.mult, op1=ALU.add)

            # ---- QKV ----
            nc.vector.memset(v_aug[:], 0.0)
            ones_col = nc.const_aps.tensor(1.0, (128, NJ, 2, 1), bf16)
            nc.vector.tensor_copy(v_aug[:, :, :, 0:1], ones_col)
            with tc.tile_pool(name="qkvps", bufs=3, space="PSUM") as qkvps:
                for t in range(8):
                    for (w_t, b_t, dst) in ((wqr, bq, qTr), (wkr, bk, kTr)):
                        ps = qkvps.tile([128, NI], f32, tag="qk", name="qk")
                        nc.tensor.matmul(ps[:], lhsT=w_t[:, 0:128],
                                         rhs=xnr0[:, ts(t, NI)],
                                         start=True, stop=False)
                        nc.tensor.matmul(ps[:], lhsT=w_t[:, 128:256],
                                         rhs=xnr1[:, ts(t, NI)],
                                         start=False, stop=True)
                        nc.vector.tensor_scalar(out=dst[:, ts(t, NI)],
                                                in0=ps[:], scalar1=b_t[:],
                                                scalar2=None, op0=ALU.add)
                for t in range(NJ):
                    vp = qkvps.tile([128, 128], f32, tag="v", name="v")
                    nc.tensor.matmul(vp[:], lhsT=xnr0[:, ts(t, 128)],
                                     rhs=wvr[:, 0:128], start=True, stop=False)
                    nc.tensor.matmul(vp[:], lhsT=xnr1[:, ts(t, 128)],
                                     rhs=wvr[:, 128:256], start=False, stop=True)
                    nc.vector.tensor_copy(
                        v_aug[:, t, :, 64:128],
                        vp[:].rearrange("p (h d) -> p h d", h=2))

        # ---- attention ----
        with tc.tile_pool(name="attsb", bufs=1) as attsb, \
             tc.tile_pool(name="spool", bufs=2, space="PSUM") as spool, \
             tc.tile_pool(name="avpool", bufs=4, space="PSUM") as avpool, \
             tc.tile_pool(name="ppool", bufs=3) as ppool:
            sums0 = attsb.tile([1, 16 * NI], f32, tag="sums0")
            for ic in range(NIC):
                av = [avpool.tile([128, NI], f32, tag="av", name=f"av{ic}_{h}")
                      for h in (0, 1)]
                for j in range(NJ):
                    sp = spool.tile([128, 2 * NI], f32, tag="sp", name="sp")
                    for h in (0, 1):
                        nc.tensor.matmul(
                            sp[:, ts(h, NI)],
                            lhsT=kTr[ts(h, 64), ts(j, 128)],
                            rhs=qTr[ts(h, 64), ts(ic, NI)],
                            start=True, stop=True)
                    pt = ppool.tile([128, 2 * NI], bf16, tag="pt", name="pt")
                    nc.scalar.activation(pt[:], sp[:], ACTF.Exp, scale=0.125)
                    for h in (0, 1):
                        nc.tensor.matmul(
                            av[h][:, :],
                            lhsT=v_aug[:, j, h, :],
                            rhs=pt[:, ts(h, NI)],
                            start=(j == 0), stop=(j == NJ - 1))
                for h in (0, 1):
                    nc.vector.tensor_copy(outT[ts(h, 64), ts(ic, NI)],
                                          av[h][64:128, :])
                    r = h * NIC + ic
                    nc.vector.tensor_copy(sums0[0:1, ts(r, NI)],
                                          av[h][0:1, :])
            # softmax denominators -> 16 partitions -> reciprocal
            nc.sync.dma_start(sums16[:],
                              sums0[:].rearrange("o (p f) -> o p f", p=16))
        nc.vector.reciprocal(recip16[:], sums16[:])

        # ---- normalize (multiply by PE-broadcast 1/sums), write f32r ----
        with tc.tile_pool(name="bcps", bufs=4, space="PSUM") as bcps:
            for h in (0, 1):
                for ic in range(NIC):
                    r = h * NIC + ic
                    bc = bcps.tile([128, NI], f32, tag="bc", name="bc")
                    nc.tensor.matmul(bc[:], lhsT=e16[:, ts(r, 128)],
                                     rhs=recip16[:], start=True, stop=True)
                    nc.vector.tensor_tensor(
                        out=out_norm[ts(h, 64), ts(ic, NI)],
                        in0=outT[ts(h, 64), ts(ic, NI)],
                        in1=bc[ts(h, 64), :], op=ALU.mult)

        # ---- projection (partial over this core's 128-channel slice) ----
        with tc.tile_pool(name="pps", bufs=4, space="PSUM") as pps, \
             tc.tile_pool(name="pstage", bufs=4) as pstage:
            for oc in (0, 1):
                for t in range(8):
                    pp = pps.tile([128, NI], f32, tag="pp", name="pp")
                    nc.tensor.matmul(pp[:], lhsT=wpr[:, ts(oc, 128)],
                                     rhs=out_norm[:, ts(t, NI)],
                                     start=True, stop=True)
                    st = pstage.tile([128, NI], f32, tag="st", name="st")
                    nc.vector.tensor_copy(st[:], pp[:])
                    nc.sync.dma_start(out_d[ts(oc, 128), ts(t, NI)], st[:])

    nc.compile()
    return nc


def _host_inputs(x, gn_scale, gn_bias, qkv_w, qkv_b, proj_w):
    """Per-core input dicts + per-core constant corrections."""
    x = np.ascontiguousarray(np.asarray(x, dtype=np.float32))
    gn_scale = np.asarray(gn_scale, dtype=np.float32)
    gn_bias = np.asarray(gn_bias, dtype=np.float32)
    qkv_w = np.asarray(qkv_w, dtype=np.float32)
    qkv_b = np.asarray(qkv_b, dtype=np.float32)
    proj_w = np.asarray(proj_w, dtype=np.float32)

    def dev_wT(WT):  # [256, 128] -> [128, 256] with free = (chunk, col)
        return np.ascontiguousarray(
            WT.reshape(2, 128, 128).transpose(1, 0, 2).reshape(128, 256))

    sel = np.zeros((128, 64), np.float32)
    rep = np.zeros((32, 256), np.float32)
    for p in range(128):
        sel[p, p // 8] = 1.0 / 8
        sel[p, 32 + 16 + p // 8] = 1.0 / 8
        rep[p // 8, p] = 1.0
        rep[16 + p // 8, 128 + p] = 1.0
    e16 = np.ascontiguousarray(
        np.repeat(np.eye(16, dtype=np.float32), 128, axis=1))

    in_maps = []
    corrs = []
    for core in range(N_CORES):
        s, hg = core // 2, core % 2
        H0, H1 = 2 * hg, 2 * hg + 1
        rows = np.r_[H0 * D:(H0 + 1) * D, H1 * D:(H1 + 1) * D]
        wq = dev_wT(np.concatenate(
            [qkv_w[0 * C + H0 * D:0 * C + (H0 + 1) * D].T,
             qkv_w[0 * C + H1 * D:0 * C + (H1 + 1) * D].T], axis=1))
        wk = dev_wT(np.concatenate(
            [qkv_w[C + H0 * D:C + (H0 + 1) * D].T,
             qkv_w[C + H1 * D:C + (H1 + 1) * D].T], axis=1))
        wv = dev_wT(np.concatenate(
            [qkv_w[2 * C + H0 * D:2 * C + (H0 + 1) * D].T,
             qkv_w[2 * C + H1 * D:2 * C + (H1 + 1) * D].T], axis=1))
        wp = np.ascontiguousarray(proj_w[:, rows].T)
        bq = np.concatenate([qkv_b[H0 * D:(H0 + 1) * D],
                             qkv_b[H1 * D:(H1 + 1) * D]]).reshape(128, 1)
        bk = np.concatenate([qkv_b[C + H0 * D:C + (H0 + 1) * D],
                             qkv_b[C + H1 * D:C + (H1 + 1) * D]]).reshape(128, 1)
        bv = np.concatenate([qkv_b[2 * C + H0 * D:2 * C + (H0 + 1) * D],
                             qkv_b[2 * C + H1 * D:2 * C + (H1 + 1) * D]])
        corrs.append(proj_w[:, rows] @ bv)  # constant v-bias correction
        in_maps.append({
            "x_s": np.ascontiguousarray(x[s].reshape(C, HW)),
            "wq": wq, "wk": wk, "wv": wv, "wp": wp,
            "bq": np.ascontiguousarray(bq), "bk": np.ascontiguousarray(bk),
            "gnsc": np.ascontiguousarray(gn_scale.reshape(2, 128).T),
            "gnbi": np.ascontiguousarray(gn_bias.reshape(2, 128).T),
            "sel": sel, "rep": rep, "e16": e16,
        })
    return x, in_maps, corrs


def kernel(x, gn_scale, gn_bias, qkv_w, qkv_b, proj_w, proj_b, _trace=False):
    from concourse import bass_utils

    if "nc" not in _cache:
        _cache["nc"] = _build_module()
    nc = _cache["nc"]

    x, in_maps, corrs = _host_inputs(x, gn_scale, gn_bias, qkv_w, qkv_b, proj_w)
    proj_b = np.asarray(proj_b, dtype=np.float32)

    res = bass_utils.run_bass_kernel_spmd(
        nc, in_maps, core_ids=list(range(N_CORES)), trace=_trace)
    _cache["last_result"] = res

    out = np.empty((B, C, Hs, Ws), np.float32)
    for s in range(B):
        acc = x[s].reshape(C, HW).copy()
        acc += res.results[2 * s]["outp"] + res.results[2 * s + 1]["outp"]
        acc += (proj_b + corrs[2 * s] + corrs[2 * s + 1])[:, None]
        out[s] = acc.reshape(C, Hs, Ws)
    return out
